# revision 19
# baseline (speedup 1.0000x reference)
"""NodeVarGraphConvolutionLayer on 8 TRN2 NeuronCores.

Math (see reference):
  Xs = X.sum(-1)                        [B, N]
  P0 = Xs;  P_i = A @ P_{i-1}           (3 batched matvecs, N=1024)
  Y[b,n,c] = sum_i h[i,c,n] * P_i[b,n]  [B, N, 64]
  out = tanh(LayerNorm_c(Y))            (gamma=1, beta=0 folded away)

Sharding: data-parallel over batch. B=16 -> 2 batches per core.

v5 design (v4 measured 55.3us; trace showed: PE cold (HAM 1.2GHz) until
29.5us, A DMA trickling until 23us, and a 15us serial epilogue tail):
  * fp16 matvec chain on A/32 (host-scaled) as in v4.  fp8 was measured
    (host sim) at rel err 0.26 -- the LN sign-flip mechanism amplifies
    chain error ~sqrt(eps); fp16+bf16 taps give 9.3e-3 vs the 2e-2 gate.
  * DMA: all of A goes on the fast SWDGE ring in strict priority order
    (b0's four 512KB chunks, then b1's, then the H blob); the tiny
    BX+BF blob rides the scalar HWDGE ring first.  dma_starts are the
    first instructions so descriptors hit the rings ~1us into the body.
  * PE warm-up: a dozen junk matmuls on a memset tile (zero DMA deps)
    run while A streams in, so HAM un-throttles (1.2->2.4GHz) before the
    first real matvec; chunk-granular psum accumulation consumes A as
    it lands.
  * Epilogue in [p, c, t] layout (t innermost, stride 1) so the big
    elementwise Y ops can hit the DVE 2x perf mode; h is host-packed as
    HBT[p,i,c,t].  LN stats come from host moments (BF blob) on the 4
    chain taps; stats partials for taps 0-2 are precomputed mid-chain so
    only the tap-3 terms remain after the last transpose.  rstd = Quake
    seed + 1 Newton step.
  * b0's entire epilogue (taps, stats, yfin, tanh, OUT DMA on sync)
    overlaps b1's chain; b1's last pass is emitted q-half-split so its
    transpose/epilogue pipeline starts half a pass early.
"""

import numpy as np

B, N, C, K1 = 16, 1024, 64, 4
NCORES = 8
BPC = B // NCORES  # batches per core
LN_EPS = 1e-5
JUNK_MM = 12  # PE warm-up matmuls (~4-5us cold)

_NC = None


def _build_module():
    from concourse import bacc, bass, tile, mybir

    f32 = mybir.dt.float32
    bf16 = mybir.dt.bfloat16
    f16 = mybir.dt.float16
    i32 = mybir.dt.int32
    AX = mybir.AxisListType
    OP = mybir.AluOpType
    AF = mybir.ActivationFunctionType

    nc = bacc.Bacc(
        "TRN2",
        target_bir_lowering=False,
        debug=False,
        enable_asserts=False,
    )

    # A^T chunked: AT16[b, c, p, jj, n] = A[b, n, 128*(2c+jj)+p] / 32
    AT_d = nc.dram_tensor(
        "AT16", [BPC, 4, 128, 2, N], f16, kind="ExternalInput"
    ).ap()
    # BXBF[p, 0:16]: Xs fp16 per (b, t); [16:32]: Xs bf16 bit patterns;
    # [32]: EYE column (1.0 at partitions 0/64); [33]: pad;
    # [34:354]: BF moments (fp32 as fp16 bit pairs): M2 [t,16], HM [t,4]
    i16 = mybir.dt.int16
    BXBF_d = nc.dram_tensor("BXBF", [128, 354], i16, kind="ExternalInput").ap()
    # HBT[p, i, c, t] = h[i, c, 128t+p]
    HBT_d = nc.dram_tensor("HBT", [128, K1, C, 8], bf16, kind="ExternalInput").ap()
    # OUT[b, p, c, t] -> host un-permutes to [b, 128t+p, c]
    OUT_d = nc.dram_tensor("OUT", [BPC, 128, C, 8], f32, kind="ExternalOutput").ap()

    with tile.TileContext(nc) as tc:
        with (
            tc.tile_pool(name="big", bufs=2) as big,
            tc.tile_pool(name="aux", bufs=1) as aux,
            tc.tile_pool(name="psum", bufs=2, space="PSUM") as psum,
            tc.tile_pool(name="psum1", bufs=1, space="PSUM") as psum1,
        ):
            # ---- DMA plan: everything big on SWDGE in priority order.
            BXBF_sb = aux.tile([128, 354], i16, tag="BXBF")
            nc.scalar.dma_start(BXBF_sb, BXBF_d)

            A_tiles = [
                [
                    aux.tile([128, 2, N], f16, tag=f"A{b}c{c}", name=f"A{b}c{c}")
                    for c in range(4)
                ]
                for b in range(BPC)
            ]
            HBT_sb = aux.tile([128, K1, C, 8], bf16, tag="HBT")
            # All bulk data on the SWDGE ring in priority order (~330
            # GB/s sustained; the HWDGE rings crawl at ~50 GB/s when
            # SWDGE is busy, so they only carry BXBF and the OUT tail).
            for b in range(BPC):
                for c in range(4):
                    nc.gpsimd.dma_start(A_tiles[b][c], AT_d[b, c])
            nc.gpsimd.dma_start(HBT_sb, HBT_d)

            # ---- init tiles (no DMA deps) + PE warm-up junk matmuls
            zero_sb = aux.tile([128, 1], f32, tag="zero")
            nc.vector.memset(zero_sb, 0.0)
            magic = aux.tile([128, 8], i32, tag="magic")
            nc.vector.memset(magic, 0x5F3759DF)
            zerob_sb = aux.tile([128, 1], bf16, tag="zerob")
            nc.vector.memset(zerob_sb, 0.0)
            junk_sb = aux.tile([128, 512], f16, tag="junk")
            nc.vector.memset(junk_sb, 0.5)
            # Preload the Tanh ACT table while DMAs run.
            warm = aux.tile([128, 1], f32, tag="warm")
            nc.scalar.activation(warm, zero_sb, AF.Tanh, bias=zero_sb)

            junk_ps = psum1.tile([1, 512], f32, tag="junkps")
            prPs = [
                psum1.tile([65, 512], f32, tag=f"prP{b}", name=f"prP{b}")
                for b in range(BPC)
            ]
            for b in range(BPC):
                # init partitions 1-63 once so the single [65,512] s2
                # copy never reads uninitialized psum
                nc.vector.memset(prPs[b], 0.0)
            for k in range(JUNK_MM):
                nc.tensor.matmul(
                    junk_ps,
                    junk_sb[:, 0:1],
                    junk_sb,
                    start=(k == 0),
                    stop=(k == JUNK_MM - 1),
                )

            # ---- views into the BXBF blob
            BXf16 = BXBF_sb.bitcast(f16)
            Xs16_v = BXf16[:, 0:16].rearrange("p (b t) -> p b t", b=BPC)
            Xsbf_v = (
                BXf16[:, 16:32].rearrange("p (b t) -> p b t", b=BPC).bitcast(bf16)
            )
            EYE_v = BXf16[0:65, 32:33]
            BF_v = BXBF_sb[:, 34:354].bitcast(f32)  # [128, 160]
            M2_v = BF_v[:, 0:128].rearrange("p (t z) -> p t z", t=8, z=16)
            HM_v = BF_v[:, 128:160].rearrange("p (t z) -> p t z", t=8, z=K1)

            # ---- chain state
            # colmm: fp16 chain values (P_i/32^i), 2-elem padded for 4B
            # alignment of the [128,1] stationary slices.
            # cole_T[p, i, t] = P_i[128t+p] in bf16 (un-scaled); the Y
            # epilogue reads [p, t] slices (t stride 1 -> DVE 2x mode),
            # the stats read it via a permuted [p, t, i] AP view.
            colmms = []
            coleTs = []
            for b in range(BPC):
                colmm = big.tile(
                    [128, 8, K1, 2], f16, tag=f"colmm{b}", name=f"colmm{b}"
                )
                coleT = big.tile([128, K1, 8], bf16, tag=f"coleT{b}", name=f"coleT{b}")
                nc.vector.tensor_copy(colmm[:, :, 0, 0], Xs16_v[:, b])
                nc.vector.tensor_copy(coleT[:, 0, :], Xsbf_v[:, b])
                colmms.append(colmm)
                coleTs.append(coleT)

            Yaccs = [
                big.tile([128, C, 8], bf16, tag=f"Yacc{b}", name=f"Yacc{b}")
                for b in range(BPC)
            ]
            Ytmps = [
                big.tile([128, C, 8], bf16, tag=f"Ytmp{b}", name=f"Ytmp{b}")
                for b in range(BPC)
            ]
            Yns = [
                big.tile([128, C, 8], bf16, tag=f"Yn{b}", name=f"Yn{b}")
                for b in range(BPC)
            ]
            s2s = [
                big.tile([65, 512], f16, tag=f"s2{b}", name=f"s2{b}")
                for b in range(BPC)
            ]

            def coleb(b, i, cl=None, ch=None):
                # [p, t] tap slice broadcast along c: stride-0 middle dim
                cl = 0 if cl is None else cl
                ch = C if ch is None else ch
                return coleTs[b][:, i : i + 1, :].broadcast_to([128, ch - cl, 8])

            def Hv(b, i, cl=None, ch=None):
                cl = 0 if cl is None else cl
                ch = C if ch is None else ch
                return HBT_sb[:, i, cl:ch]

            # ---- pipeline pieces
            def junk(n):
                # PE keep-warm filler: no data deps, keeps the HAM busy
                # window covered while DMA chunks land.
                for _ in range(n):
                    nc.tensor.matmul(junk_ps, junk_sb[:, 0:1], junk_sb)

            def accpart(i, b, cseq, pr, start, stop):
                # matvec pass i for batch b: psum rows at partitions 0/64;
                # emitted per chunk-group so the PE FIFO follows DMA arrival.
                colmm = colmms[b]
                pairs = [(c, jj) for c in cseq for jj in range(2)]
                for nj, (c, jj) in enumerate(pairs):
                    for q in range(2):
                        nc.tensor.matmul(
                            pr[64 * q : 64 * q + 1, :],
                            colmm[:, 2 * c + jj, i - 1, 0:1],
                            A_tiles[b][c][:, jj, 512 * q : 512 * (q + 1)],
                            start=(start and nj == 0),
                            stop=(stop and nj == len(pairs) - 1),
                        )

            def acc(i, b):
                pr = prPs[b]
                accpart(i, b, (0, 1, 2, 3), pr, True, True)
                return pr

            def s2copy(b, pr, q, eng=None):
                eng = eng or nc.scalar
                with tc.high_priority():
                    eng.tensor_copy(
                        s2s[b][64 * q : 64 * q + 1, :], pr[64 * q : 64 * q + 1, :]
                    ) if eng is nc.vector else eng.copy(
                        s2s[b][64 * q : 64 * q + 1, :], pr[64 * q : 64 * q + 1, :]
                    )

            def s2copy1(b, pr):
                # single ACT copy of both psum rows (partitions 1-63 are
                # never-written garbage that s2's readers never touch);
                # same duration as one row (partitions process in parallel)
                with tc.high_priority():
                    nc.scalar.copy(s2s[b], pr)

            def ptmm(i, b, q):
                # fp16 K=1 outer-product transposes: 4 MMs per q-half
                with tc.high_priority():
                    pt = pts[b]
                    for u in range(4):
                        nc.tensor.matmul(
                            pt[:, 4 * q + u, 0:1],
                            s2s[b][64 * q : 64 * q + 1, 128 * u : 128 * (u + 1)],
                            EYE_v[64 * q : 64 * q + 1, :],
                            is_transpose=True,
                            start=(u == 0),
                            stop=(u == 3),
                        )

            def ptout(i, b, q=None):
                # pt -> colmm (next stationary) + cole_T (bf16 tap)
                with tc.high_priority():
                    pt = pts[b]
                    if q is None:
                        tsl = slice(0, 8)
                    else:
                        tsl = slice(4 * q, 4 * q + 4)
                    if i < K1 - 1:
                        nc.scalar.copy(colmms[b][:, tsl, i, 0], pt[:, tsl, 0])
                    nc.scalar.activation(
                        coleTs[b][:, i, tsl],
                        pt[:, tsl, 0],
                        AF.Copy,
                        scale=float(32.0**i),
                    )

            pts = [
                psum1.tile([128, 8, 2], f16, tag=f"pt{b}", name=f"pt{b}")
                for b in range(BPC)
            ]

            def tra(i, b, pr):
                s2copy1(b, pr)
                for q in range(2):
                    ptmm(i, b, q)
                ptout(i, b)

            # taps: Yacc = sum_i h_i * c_i, built incrementally
            def tap01(b, eng):
                eng.tensor_tensor(Yaccs[b], Hv(b, 0), coleb(b, 0), OP.mult)
                eng.tensor_tensor(Ytmps[b], Hv(b, 1), coleb(b, 1), OP.mult)
                eng.tensor_tensor(Yaccs[b], Yaccs[b], Ytmps[b], OP.add)

            def tap(b, i, eng, cl=None, ch=None):
                cl_ = 0 if cl is None else cl
                ch_ = C if ch is None else ch
                sl = slice(cl_, ch_)
                eng.tensor_tensor(
                    Ytmps[b][:, sl], Hv(b, i, cl_, ch_), coleb(b, i, cl_, ch_), OP.mult
                )
                eng.tensor_tensor(
                    Yaccs[b][:, sl], Yaccs[b][:, sl], Ytmps[b][:, sl], OP.add
                )

            # ---- LN stats from host moments; partials for taps 0-2
            # mid-chain, tap-3 terms + Quake rsqrt in the tail.
            stats_state = {}

            def statspart(b):
                col = coleTs[b].rearrange("p i t -> p t i")  # [128, 8, 4]
                c012 = col[:, :, 0:3]
                cc = big.tile([128, 8, 3, 3], f32, tag=f"cc{b}")
                nc.vector.tensor_tensor(
                    cc,
                    c012.unsqueeze(3).broadcast_to([128, 8, 3, 3]),
                    c012.unsqueeze(2).broadcast_to([128, 8, 3, 3]),
                    OP.mult,
                )
                m2t = big.tile([128, 8, 3, 3], f32, tag=f"m2t{b}")
                # M2 submatrix [0:3, 0:3] of the 4x4: rows i<3, cols j<3
                M2sub = M2_v.rearrange("p t (i j) -> p t i j", i=4)[:, :, 0:3, 0:3]
                nc.vector.tensor_tensor(m2t, cc, M2sub, OP.mult)
                ey2p = big.tile([128, 8], f32, tag=f"ey2p{b}")
                nc.vector.tensor_reduce(ey2p, m2t, AX.XY, OP.add)

                mm3 = big.tile([128, 8, 3], f32, tag=f"mm3{b}")
                nc.vector.tensor_tensor(mm3, c012, HM_v[:, :, 0:3], OP.mult)
                mup = big.tile([128, 8], f32, tag=f"mup{b}")
                nc.vector.tensor_reduce(mup, mm3, AX.X, OP.add)
                stats_state[b] = (ey2p, mup)

            def statsfin(b):
                # M2W row-3 host-packed: [2*M2[3,0], 2*M2[3,1], 2*M2[3,2],
                # M2[3,3]] so ey2 = ey2p + sum_j M2W_j * c3 * c_j
                ey2p, mup = stats_state[b]
                col = coleTs[b].rearrange("p i t -> p t i")
                c3 = col[:, :, 3:4].broadcast_to([128, 8, 4])
                cc3 = big.tile([128, 8, 4], f32, tag=f"cc3{b}")
                nc.vector.tensor_tensor(cc3, c3, col, OP.mult)
                M2W = M2_v.rearrange("p t (i j) -> p t i j", i=4)[:, :, 3, :]
                m23 = big.tile([128, 8, 4], f32, tag=f"m23{b}")
                nc.vector.tensor_tensor(m23, cc3, M2W, OP.mult)
                ey23 = big.tile([128, 8], f32, tag=f"ey23{b}")
                nc.vector.tensor_reduce(ey23, m23, AX.X, OP.add)
                ey2 = big.tile([128, 8], f32, tag=f"ey2{b}")
                nc.vector.tensor_tensor(ey2, ey2p, ey23, OP.add)

                mm1 = big.tile([128, 8], f32, tag=f"mm1{b}")
                nc.vector.tensor_tensor(mm1, col[:, :, 3], HM_v[:, :, 3], OP.mult)
                mu = big.tile([128, 8], f32, tag=f"mu{b}")
                nc.vector.tensor_tensor(mu, mup, mm1, OP.add)
                mu2 = big.tile([128, 8], f32, tag=f"mu2{b}")
                nc.vector.tensor_tensor(mu2, mu, mu, OP.mult)
                veps = big.tile([128, 8], f32, tag=f"veps{b}")
                nc.vector.tensor_tensor(veps, ey2, mu2, OP.subtract)

                # Quake rsqrt + 1 Newton iteration (eps dropped: var >> 1)
                rstd = big.tile([128, 8], f32, tag=f"rstd{b}")
                nc.vector.tensor_scalar(
                    rstd.bitcast(i32), veps.bitcast(i32), 1, None,
                    OP.logical_shift_right,
                )
                nc.vector.tensor_tensor(
                    rstd.bitcast(i32), magic, rstd.bitcast(i32), OP.subtract
                )
                tq = big.tile([128, 8], f32, tag=f"tq{b}")
                nc.vector.tensor_tensor(tq, rstd, rstd, OP.mult)
                nc.vector.scalar_tensor_tensor(tq, tq, -0.5, veps, OP.mult, OP.mult)
                nc.vector.scalar_tensor_tensor(rstd, tq, 1.5, rstd, OP.add, OP.mult)

                mur = big.tile([128, 8], f32, tag=f"mur{b}")
                nc.vector.tensor_tensor(mur, mu, rstd, OP.mult)
                return rstd, mur

            def yfin(b, rstdh, murh, eng, cl, ch):
                sl = slice(cl, ch)
                rb = rstdh.unsqueeze(1).broadcast_to([128, ch - cl, 8])
                mb = murh.unsqueeze(1).broadcast_to([128, ch - cl, 8])
                eng.tensor_tensor(Yns[b][:, sl], Yaccs[b][:, sl], rb, OP.mult)
                eng.tensor_tensor(Yns[b][:, sl], Yns[b][:, sl], mb, OP.subtract)

            def outwrite(b, ring, bias, split=False):
                OUT_sb = big.tile([128, C, 8], f32, tag=f"OUTS{b}")
                for half in range(2):
                    sl = slice(32 * half, 32 * half + 32)
                    nc.scalar.activation(
                        OUT_sb[:, sl], Yns[b][:, sl], AF.Tanh, bias=bias
                    )
                    if split:
                        ring.dma_start(OUT_d[b][:, sl], OUT_sb[:, sl])
                if not split:
                    ring.dma_start(OUT_d[b], OUT_sb)

            # ---- emission in planned execution order ----
            # b0's chain is scheduled to COMPLETE before b1's A lands
            # (b0's A arrives ~6us earlier); b1's chunk-groups and the
            # pt transposes fill the tra-latency gaps.  b0's epilogue
            # then drains on DVE/GPSIMD well before b1's tail starts.
            pr0, pr1 = prPs[0], prPs[1]
            junk(16)
            accpart(1, 0, (0,), pr0, True, False)
            junk(1)
            accpart(1, 0, (1,), pr0, False, False)
            junk(1)
            accpart(1, 0, (2,), pr0, False, False)
            junk(1)
            accpart(1, 0, (3,), pr0, False, True)
            s2copy1(0, pr0)
            ptmm(1, 0, 0)
            ptmm(1, 0, 1)
            ptout(1, 0)
            accpart(2, 0, (0, 1, 2, 3), pr0, True, True)
            accpart(1, 1, (0,), pr1, True, False)
            s2copy1(0, pr0)  # s2(2,0)
            ptmm(2, 0, 0)
            ptmm(2, 0, 1)
            ptout(2, 0)
            accpart(1, 1, (1,), pr1, False, False)
            accpart(3, 0, (0, 1, 2, 3), pr0, True, True)
            accpart(1, 1, (2,), pr1, False, False)
            s2copy1(0, pr0)  # s2(3,0)
            ptmm(3, 0, 0)
            ptmm(3, 0, 1)
            ptout(3, 0)
            accpart(1, 1, (3,), pr1, False, True)
            s2copy1(1, pr1)  # s2(1,1)
            ptmm(1, 1, 0)
            ptmm(1, 1, 1)
            ptout(1, 1)
            # DVE/GPSIMD epilogue streams (emission = FIFO order)
            tap01(0, nc.vector)
            statspart(0)
            tap(0, 2, nc.gpsimd)
            tap(0, 3, nc.gpsimd)
            accpart(2, 1, (0, 1, 2, 3), pr1, True, True)
            s2copy1(1, pr1)  # s2(2,1)
            ptmm(2, 1, 0)
            ptmm(2, 1, 1)
            ptout(2, 1)
            r0 = statsfin(0)
            yfin(0, *r0, nc.vector, 0, C)
            tap01(1, nc.vector)
            statspart(1)
            accpart(3, 1, (0, 1, 2, 3), pr1, True, True)
            tap(1, 2, nc.gpsimd)
            s2copy1(1, pr1)  # s2(3,1)
            ptmm(3, 1, 0)
            ptmm(3, 1, 1)
            ptout(3, 1)
            # tanh(b0) bias derives (=0) from the last cole copy so the
            # scheduler cannot slot it ahead of the pipeline ACT copies.
            biasb0 = big.tile([128, 1], bf16, tag="biasb0")
            nc.vector.tensor_scalar_mul(biasb0, coleTs[1][:, 3, 0:1], 0.0)
            outwrite(0, nc.sync, biasb0)
            # b1 tail: tap3 split across DVE (ahead of stats) + GPSIMD
            tap(1, 3, nc.vector, 0, 32)
            tap(1, 3, nc.gpsimd, 32, C)
            r1 = statsfin(1)
            yfin(1, *r1, nc.vector, 0, 32)
            yfin(1, *r1, nc.gpsimd, 32, C)
            outwrite(1, nc.scalar, zerob_sb, split=True)

    nc.compile()
    return nc


def _get_module():
    global _NC
    if _NC is None:
        _NC = _build_module()
    return _NC


def _make_in_maps(A, X, h):
    import ml_dtypes

    bf16 = ml_dtypes.bfloat16
    # AT16[b, c, p, jj, n] = A[b, n, 128*(2c+jj)+p] / 32
    AT = A.transpose(0, 2, 1).reshape(B, 4, 2, 128, N).transpose(0, 1, 3, 2, 4)
    AT16 = (AT / np.float32(32.0)).astype(np.float16)

    Xs = X.astype(np.float32).sum(-1)  # [B, N] (fp32: the fp16 pre-cast cost ~6% of the error budget)
    Xs16 = Xs.astype(np.float16).reshape(B, 8, 128)
    Xsbf = Xs.astype(bf16).view(np.uint16).reshape(B, 8, 128)

    # HBT[p, i, c, t] = h[i, c, 128t+p]
    HBT = np.ascontiguousarray(
        h.reshape(K1, C, 8, 128).transpose(3, 0, 1, 2)
    ).astype(bf16)

    # Host LN moments: HM[n, i] = mean_c h[i,c,n]; M2[n, i*4+j] = mean_c h_i h_j
    # Row 3 of M2 is pre-weighted for the tail: [2*M2_30, 2*M2_31, 2*M2_32, M2_33]
    hf = h.astype(np.float64)
    HMF = hf.mean(axis=1).T.astype(np.float32)  # [N, K1]
    M2F = (np.einsum("icn,jcn->nij", hf, hf) / C).astype(np.float32)  # [N, 4, 4]
    M2F[:, 3, 0:3] *= 2.0
    M2F = M2F.reshape(N, K1 * K1)
    BF = np.concatenate(
        [
            M2F.reshape(8, 128, 16).transpose(1, 0, 2).reshape(128, 128),
            HMF.reshape(8, 128, K1).transpose(1, 0, 2).reshape(128, 32),
        ],
        axis=1,
    )
    BF = np.ascontiguousarray(BF, dtype=np.float32)

    in_maps = []
    for core in range(NCORES):
        sl = slice(BPC * core, BPC * (core + 1))
        BXBF = np.zeros((128, 354), dtype=np.float16)
        BXBF[:, 0:16] = Xs16[sl].transpose(2, 0, 1).reshape(128, 16)
        BXBF[:, 16:32] = (
            Xsbf[sl].transpose(2, 0, 1).reshape(128, 16).view(np.float16)
        )
        BXBF[0, 32] = 1.0
        BXBF[64, 32] = 1.0
        BXBF = BXBF.view(np.int16).copy()
        BXBF[:, 34:354] = BF.view(np.int16)
        in_maps.append(
            {
                "AT16": np.ascontiguousarray(AT16[sl]),
                "BXBF": np.ascontiguousarray(BXBF),
                "HBT": HBT,
            }
        )
    return in_maps


def _unpermute_out(raw):
    # raw [BPC, 128, C, 8] -> [BPC, N, C] with n = 128t + p
    return np.ascontiguousarray(
        np.asarray(raw).transpose(0, 3, 1, 2).reshape(BPC, N, C)
    )


def _numpy_fallback(A, X, h, ln_gamma, ln_beta):
    Xs = X.sum(-1)
    p = Xs
    powers = [Xs]
    for _ in range(K1 - 1):
        p = np.einsum("bnm,bm->bn", A, p)
        powers.append(p)
    P = np.stack(powers)
    Y = np.einsum("icn,ibn->bnc", h, P)
    mu = Y.mean(axis=-1, keepdims=True)
    var = Y.var(axis=-1, keepdims=True)
    Yn = (Y - mu) / np.sqrt(var + LN_EPS) * ln_gamma + ln_beta
    return np.tanh(Yn).astype(np.float32)


def _run(A, X, h, ln_gamma, ln_beta, trace=False):
    A = np.ascontiguousarray(np.asarray(A, dtype=np.float32))
    X = np.ascontiguousarray(np.asarray(X, dtype=np.float32))
    h = np.ascontiguousarray(np.asarray(h, dtype=np.float32))
    g = np.asarray(ln_gamma, dtype=np.float32)
    be = np.asarray(ln_beta, dtype=np.float32)

    if not (np.all(g == 1.0) and np.all(be == 0.0)):
        # device kernel folds the (identity) affine away; anything else is
        # handled on host
        return _numpy_fallback(A, X, h, g, be), None

    from concourse import bass_utils

    nc = _get_module()
    res = bass_utils.run_bass_kernel_spmd(
        nc, _make_in_maps(A, X, h), core_ids=list(range(NCORES)), trace=trace
    )
    out = np.concatenate(
        [_unpermute_out(r["OUT"]) for r in res.results], axis=0
    )
    return out.astype(np.float32, copy=False), res.exec_time_ns


def kernel(A, X, h, ln_gamma, ln_beta):
    out, _ = _run(A, X, h, ln_gamma, ln_beta, trace=False)
    return out


def kernel_profiled(A, X, h, ln_gamma, ln_beta):
    return _run(A, X, h, ln_gamma, ln_beta, trace=True)


# revision 31
# speedup vs baseline: 1.0655x; 1.0655x over previous
"""NodeVarGraphConvolutionLayer on 8 TRN2 NeuronCores.

Math (see reference):
  Xs = X.sum(-1)                        [B, N]
  P0 = Xs;  P_i = A @ P_{i-1}           (3 batched matvecs, N=1024)
  Y[b,n,c] = sum_i h[i,c,n] * P_i[b,n]  [B, N, 64]
  out = tanh(LayerNorm_c(Y))            (gamma=1, beta=0 folded away)

Sharding: data-parallel over batch. B=16 -> 2 batches per core.

v6 design (v4 baseline measured 55.3us; now ~46us median, rel err
8.9e-3 vs the 2e-2 gate):
  * fp16 matvec chain on A/32 (host-scaled) as in v4.  fp8 DoubleRow
    was host-simulated at rel err 0.26 -- LN sign-flips amplify chain
    error ~sqrt(eps) -- so fp16 stays.  Xs is summed in fp32 on host
    (the old fp16 pre-cast cost ~6% of the error budget).
  * DMA: ALL bulk data (A chunks then the H blob) on the one fast
    SWDGE ring in strict priority order: b0's four 512KB chunks, b1's,
    HBT.  A second ring only steals from the same ~358 GB/s HBM budget
    (measured), so only the tiny BXBF blob and OUT ride HWDGE rings.
  * PE warm-up: ~19 junk matmuls on a memset tile (no DMA deps) keep
    the HAM activity window busy from ~3us so the clock is at 2.4GHz
    before the first real matvec; 1-junk fillers cover the chunk
    cadence gaps.  Matvec MMs consume A chunks as they land (psum
    accumulation groups split per chunk-group in FIFO arrival order).
  * b0's whole chain is scheduled to finish before b1's A lands, so
    b0's epilogue drains on DVE/GPSIMD mid-chain and only b1's compact
    tail (tap3 + stats + yfin + tanh + OUT) follows the last transpose.
  * tra: ONE [65,512] ACT copy moves both psum rows (partitions 1-63
    are memset-once garbage that is never read); fp16 K=1 outer-product
    transposes into a psum pt tile; colmm (next stationary) is copied
    before cole (the bf16 tap) since only colmm gates the next pass.
  * Epilogue in [p, c, t] layout (t innermost); h host-packed as
    HBT[p,i,c,t].  LN stats use host COVARIANCE moments (KV = M2 -
    HM HM^T) so var is one quadratic form over the 4 taps -- no mu^2
    step; rstd = Quake seed + 1 Newton (ACT Sqrt/Rsqrt would thrash
    the 2 ACT table slots mid-kernel; DVE pow is rejected by the
    backend).  tanh(b0)'s bias derives (=0) from the last cole copy so
    the list scheduler cannot slot it ahead of pipeline ACT copies
    (measured 3.5us ACT head-of-line stall without it).
"""

import numpy as np

B, N, C, K1 = 16, 1024, 64, 4
NCORES = 8
BPC = B // NCORES  # batches per core
LN_EPS = 1e-5
JUNK_MM = 12  # PE warm-up matmuls (~4-5us cold)

_NC = None


def _build_module():
    from concourse import bacc, bass, tile, mybir

    f32 = mybir.dt.float32
    bf16 = mybir.dt.bfloat16
    f16 = mybir.dt.float16
    i32 = mybir.dt.int32
    AX = mybir.AxisListType
    OP = mybir.AluOpType
    AF = mybir.ActivationFunctionType

    nc = bacc.Bacc(
        "TRN2",
        target_bir_lowering=False,
        debug=False,
        enable_asserts=False,
    )

    # A^T chunked: AT16[b, c, p, jj, n] = A[b, n, 128*(2c+jj)+p] / 32
    AT_d = nc.dram_tensor(
        "AT16", [BPC, 4, 128, 2, N], f16, kind="ExternalInput"
    ).ap()
    # BXBF[p, 0:16]: Xs fp16 per (b, t); [16:32]: Xs bf16 bit patterns;
    # [32]: EYE column (1.0 at partitions 0/64); [33]: pad;
    # [34:354]: BF moments (fp32 as fp16 bit pairs): M2 [t,16], HM [t,4]
    i16 = mybir.dt.int16
    BXBF_d = nc.dram_tensor("BXBF", [128, 354], i16, kind="ExternalInput").ap()
    # HBT[p, i, c, t] = h[i, c, 128t+p]
    HBT_d = nc.dram_tensor("HBT", [128, K1, C, 8], bf16, kind="ExternalInput").ap()
    # OUT[b, p, c, t] -> host un-permutes to [b, 128t+p, c]
    OUT_d = nc.dram_tensor("OUT", [BPC, 128, C, 8], f32, kind="ExternalOutput").ap()

    with tile.TileContext(nc) as tc:
        with (
            tc.tile_pool(name="big", bufs=2) as big,
            tc.tile_pool(name="aux", bufs=1) as aux,
            tc.tile_pool(name="psum1", bufs=1, space="PSUM") as psum1,
        ):
            # ---- DMA plan: everything big on SWDGE in priority order.
            BXBF_sb = aux.tile([128, 354], i16, tag="BXBF")
            nc.scalar.dma_start(BXBF_sb, BXBF_d)

            A_tiles = [
                [
                    aux.tile([128, 2, N], f16, tag=f"A{b}c{c}", name=f"A{b}c{c}")
                    for c in range(4)
                ]
                for b in range(BPC)
            ]
            HBT_sb = aux.tile([128, K1, C, 8], bf16, tag="HBT")
            # All bulk data on the SWDGE ring in priority order (~350
            # GB/s sustained; the HWDGE rings are erratic -- 50-300 GB/s
            # -- so they only carry BXBF and OUT0).  Anything on a
            # second ring steals from the same ~358 GB/s HBM budget
            # anyway, so the A stream is strictly FIFO-prioritized.
            for b in range(BPC):
                for c in range(4):
                    nc.gpsimd.dma_start(A_tiles[b][c], AT_d[b, c])
            nc.gpsimd.dma_start(HBT_sb, HBT_d)

            # ---- init tiles (no DMA deps) + PE warm-up junk matmuls
            zero_sb = aux.tile([128, 1], f32, tag="zero")
            nc.vector.memset(zero_sb, 0.0)
            magic = aux.tile([128, 8], i32, tag="magic")
            nc.vector.memset(magic, 0x5F3759DF)
            zerob_sb = aux.tile([128, 1], bf16, tag="zerob")
            nc.vector.memset(zerob_sb, 0.0)
            junk_sb = aux.tile([128, 512], f16, tag="junk")
            nc.vector.memset(junk_sb, 0.5)
            # Preload the Tanh + Rsqrt ACT tables while DMAs run.
            warm = aux.tile([128, 1], f32, tag="warm")
            nc.scalar.activation(warm, zero_sb, AF.Tanh, bias=zero_sb)


            junk_ps = psum1.tile([1, 512], f32, tag="junkps")
            prPs = [
                psum1.tile([65, 512], f32, tag=f"prP{b}", name=f"prP{b}")
                for b in range(BPC)
            ]
            for b in range(BPC):
                # init partitions 1-63 once so the single [65,512] s2
                # copy never reads uninitialized psum
                nc.vector.memset(prPs[b], 0.0)
            for k in range(JUNK_MM):
                nc.tensor.matmul(
                    junk_ps,
                    junk_sb[:, 0:1],
                    junk_sb,
                    start=(k == 0),
                    stop=(k == JUNK_MM - 1),
                )

            # ---- views into the BXBF blob
            BXf16 = BXBF_sb.bitcast(f16)
            Xs16_v = BXf16[:, 0:16].rearrange("p (b t) -> p b t", b=BPC)
            Xsbf_v = (
                BXf16[:, 16:32].rearrange("p (b t) -> p b t", b=BPC).bitcast(bf16)
            )
            EYE_v = BXf16[0:65, 32:33]
            BF_v = BXBF_sb[:, 34:354].bitcast(f32)  # [128, 160]
            M2_v = BF_v[:, 0:128].rearrange("p (t z) -> p t z", t=8, z=16)
            HM_v = BF_v[:, 128:160].rearrange("p (t z) -> p t z", t=8, z=K1)

            # ---- chain state
            # colmm: fp16 chain values (P_i/32^i), 2-elem padded for 4B
            # alignment of the [128,1] stationary slices.
            # cole_T[p, i, t] = P_i[128t+p] in bf16 (un-scaled); the Y
            # epilogue reads [p, t] slices (t stride 1 -> DVE 2x mode),
            # the stats read it via a permuted [p, t, i] AP view.
            colmms = []
            coleTs = []
            for b in range(BPC):
                colmm = big.tile(
                    [128, 8, K1, 2], f16, tag=f"colmm{b}", name=f"colmm{b}"
                )
                coleT = big.tile([128, K1, 8], bf16, tag=f"coleT{b}", name=f"coleT{b}")
                nc.vector.tensor_copy(colmm[:, :, 0, 0], Xs16_v[:, b])
                nc.vector.tensor_copy(coleT[:, 0, :], Xsbf_v[:, b])
                colmms.append(colmm)
                coleTs.append(coleT)

            Yaccs = [
                big.tile([128, C, 8], bf16, tag=f"Yacc{b}", name=f"Yacc{b}")
                for b in range(BPC)
            ]
            Ytmps = [
                big.tile([128, C, 8], bf16, tag=f"Ytmp{b}", name=f"Ytmp{b}")
                for b in range(BPC)
            ]
            Yns = [
                big.tile([128, C, 8], bf16, tag=f"Yn{b}", name=f"Yn{b}")
                for b in range(BPC)
            ]
            s2s = [
                big.tile([65, 512], f16, tag=f"s2{b}", name=f"s2{b}")
                for b in range(BPC)
            ]

            def coleb(b, i, cl=None, ch=None):
                # [p, t] tap slice broadcast along c: stride-0 middle dim
                cl = 0 if cl is None else cl
                ch = C if ch is None else ch
                return coleTs[b][:, i : i + 1, :].broadcast_to([128, ch - cl, 8])

            def Hv(b, i, cl=None, ch=None):
                cl = 0 if cl is None else cl
                ch = C if ch is None else ch
                return HBT_sb[:, i, cl:ch]

            # ---- pipeline pieces
            def junk(n):
                # PE keep-warm filler: no data deps, keeps the HAM busy
                # window covered while DMA chunks land.
                for _ in range(n):
                    nc.tensor.matmul(junk_ps, junk_sb[:, 0:1], junk_sb)

            def accpart(i, b, cseq, pr, start, stop):
                # matvec pass i for batch b: psum rows at partitions 0/64;
                # emitted per chunk-group so the PE FIFO follows DMA arrival.
                colmm = colmms[b]
                pairs = [(c, jj) for c in cseq for jj in range(2)]
                for nj, (c, jj) in enumerate(pairs):
                    for q in range(2):
                        nc.tensor.matmul(
                            pr[64 * q : 64 * q + 1, :],
                            colmm[:, 2 * c + jj, i - 1, 0:1],
                            A_tiles[b][c][:, jj, 512 * q : 512 * (q + 1)],
                            start=(start and nj == 0),
                            stop=(stop and nj == len(pairs) - 1),
                        )

            def acc(i, b):
                pr = prPs[b]
                accpart(i, b, (0, 1, 2, 3), pr, True, True)
                return pr

            def s2copy1(b, pr):
                # single ACT copy of both psum rows (partitions 1-63 are
                # never-written garbage that s2's readers never touch);
                # same duration as one row (partitions process in parallel)
                with tc.high_priority():
                    nc.scalar.copy(s2s[b], pr)

            def ptmm(i, b, q):
                # fp16 K=1 outer-product transposes: 4 MMs per q-half
                with tc.high_priority():
                    pt = pts[b]
                    for u in range(4):
                        nc.tensor.matmul(
                            pt[:, 4 * q + u, 0:1],
                            s2s[b][64 * q : 64 * q + 1, 128 * u : 128 * (u + 1)],
                            EYE_v[64 * q : 64 * q + 1, :],
                            is_transpose=True,
                            start=(u == 0),
                            stop=(u == 3),
                        )

            def ptout(i, b, q=None):
                # pt -> colmm (next stationary) + cole_T (bf16 tap)
                with tc.high_priority():
                    pt = pts[b]
                    if q is None:
                        tsl = slice(0, 8)
                    else:
                        tsl = slice(4 * q, 4 * q + 4)
                    if i < K1 - 1:
                        nc.scalar.copy(colmms[b][:, tsl, i, 0], pt[:, tsl, 0])
                    nc.scalar.activation(
                        coleTs[b][:, i, tsl],
                        pt[:, tsl, 0],
                        AF.Copy,
                        scale=float(32.0**i),
                    )

            pts = [
                psum1.tile([128, 8, 2], f16, tag=f"pt{b}", name=f"pt{b}")
                for b in range(BPC)
            ]

            def tra(i, b, pr):
                s2copy1(b, pr)
                for q in range(2):
                    ptmm(i, b, q)
                ptout(i, b)

            # taps: Yacc = sum_i h_i * c_i, built incrementally
            def tap01(b, eng):
                eng.tensor_tensor(Yaccs[b], Hv(b, 0), coleb(b, 0), OP.mult)
                eng.tensor_tensor(Ytmps[b], Hv(b, 1), coleb(b, 1), OP.mult)
                eng.tensor_tensor(Yaccs[b], Yaccs[b], Ytmps[b], OP.add)

            def tap(b, i, eng, cl=None, ch=None):
                cl_ = 0 if cl is None else cl
                ch_ = C if ch is None else ch
                sl = slice(cl_, ch_)
                eng.tensor_tensor(
                    Ytmps[b][:, sl], Hv(b, i, cl_, ch_), coleb(b, i, cl_, ch_), OP.mult
                )
                eng.tensor_tensor(
                    Yaccs[b][:, sl], Yaccs[b][:, sl], Ytmps[b][:, sl], OP.add
                )

            # ---- LN stats from host COVARIANCE moments on the 4 chain
            # taps: var = sum_ij KV_ij c_i c_j (KV = M2 - HM HM^T, so no
            # mu^2 step), mu = sum_i HM_i c_i.  rstd via Quake + 1 Newton
            # with the seed NEGATED (fused shift-sub); signs cancel in
            # the Newton square and the yfin subtract order flips back.
            def statsfin(b):
                col = coleTs[b].rearrange("p i t -> p t i")
                ccF = big.tile([128, 8, 4, 4], f32, tag=f"ccF{b}")
                nc.vector.tensor_tensor(
                    ccF,
                    col.unsqueeze(3).broadcast_to([128, 8, 4, 4]),
                    col.unsqueeze(2).broadcast_to([128, 8, 4, 4]),
                    OP.mult,
                )
                KV4 = M2_v.rearrange("p t (i j) -> p t i j", i=4)
                mv = big.tile([128, 8, 4, 4], f32, tag=f"mv{b}")
                nc.vector.tensor_tensor(mv, ccF, KV4, OP.mult)
                veps = big.tile([128, 8], f32, tag=f"veps{b}")
                nc.vector.tensor_reduce(veps, mv, AX.XY, OP.add)

                mm4 = big.tile([128, 8, K1], f32, tag=f"mm4{b}")
                nc.vector.tensor_tensor(mm4, col, HM_v, OP.mult)
                mu = big.tile([128, 8], f32, tag=f"mu{b}")
                nc.vector.tensor_reduce(mu, mm4, AX.X, OP.add)

                rstd = big.tile([128, 8], f32, tag=f"rstd{b}")
                nc.vector.tensor_scalar(
                    rstd.bitcast(i32), veps.bitcast(i32), 1, None,
                    OP.logical_shift_right,
                )
                nc.vector.tensor_tensor(
                    rstd.bitcast(i32), magic, rstd.bitcast(i32), OP.subtract
                )
                tq = big.tile([128, 8], f32, tag=f"tq{b}")
                nc.vector.tensor_tensor(tq, rstd, rstd, OP.mult)
                nc.vector.scalar_tensor_tensor(tq, tq, -0.5, veps, OP.mult, OP.mult)
                nc.vector.scalar_tensor_tensor(rstd, tq, 1.5, rstd, OP.add, OP.mult)
                mur = big.tile([128, 8], f32, tag=f"mur{b}")
                nc.vector.tensor_tensor(mur, mu, rstd, OP.mult)
                return rstd, mur

            def yfin(b, rstdh, murh, eng, cl, ch):
                sl = slice(cl, ch)
                rb = rstdh.unsqueeze(1).broadcast_to([128, ch - cl, 8])
                mb = murh.unsqueeze(1).broadcast_to([128, ch - cl, 8])
                eng.tensor_tensor(Yns[b][:, sl], Yaccs[b][:, sl], rb, OP.mult)
                eng.tensor_tensor(Yns[b][:, sl], Yns[b][:, sl], mb, OP.subtract)

            def outwrite(b, ring, bias, split=False):
                OUT_sb = big.tile([128, C, 8], f32, tag=f"OUTS{b}")
                for half in range(2):
                    sl = slice(32 * half, 32 * half + 32)
                    nc.scalar.activation(
                        OUT_sb[:, sl], Yns[b][:, sl], AF.Tanh, bias=bias
                    )
                    if split:
                        ring.dma_start(OUT_d[b][:, sl], OUT_sb[:, sl])
                if not split:
                    ring.dma_start(OUT_d[b], OUT_sb)

            # ---- emission in planned execution order ----
            # b0's chain is scheduled to COMPLETE before b1's A lands
            # (b0's A arrives ~6us earlier); b1's chunk-groups and the
            # pt transposes fill the tra-latency gaps.  b0's epilogue
            # then drains on DVE/GPSIMD well before b1's tail starts.
            pr0, pr1 = prPs[0], prPs[1]
            junk(16)
            accpart(1, 0, (0,), pr0, True, False)
            junk(1)
            accpart(1, 0, (1,), pr0, False, False)
            junk(1)
            accpart(1, 0, (2,), pr0, False, False)
            junk(1)
            accpart(1, 0, (3,), pr0, False, True)
            s2copy1(0, pr0)
            ptmm(1, 0, 0)
            ptmm(1, 0, 1)
            ptout(1, 0)
            accpart(2, 0, (0, 1, 2, 3), pr0, True, True)
            accpart(1, 1, (0,), pr1, True, False)
            s2copy1(0, pr0)  # s2(2,0)
            ptmm(2, 0, 0)
            ptmm(2, 0, 1)
            ptout(2, 0)
            accpart(1, 1, (1,), pr1, False, False)
            accpart(1, 1, (2,), pr1, False, False)
            accpart(3, 0, (0, 1, 2, 3), pr0, True, True)
            accpart(1, 1, (3,), pr1, False, True)
            s2copy1(0, pr0)  # s2(3,0)
            ptmm(3, 0, 0)
            ptmm(3, 0, 1)
            ptout(3, 0)
            s2copy1(1, pr1)  # s2(1,1)
            ptmm(1, 1, 0)
            ptmm(1, 1, 1)
            ptout(1, 1)
            # DVE/GPSIMD epilogue streams (emission = FIFO order)
            tap01(0, nc.vector)
            tap(0, 2, nc.gpsimd)
            tap(0, 3, nc.gpsimd)
            accpart(2, 1, (0, 1, 2, 3), pr1, True, True)
            s2copy1(1, pr1)  # s2(2,1)
            ptmm(2, 1, 0)
            ptmm(2, 1, 1)
            ptout(2, 1)
            r0 = statsfin(0)
            yfin(0, *r0, nc.vector, 0, C)
            tap01(1, nc.vector)
            accpart(3, 1, (0, 1, 2, 3), pr1, True, True)
            tap(1, 2, nc.gpsimd)
            s2copy1(1, pr1)  # s2(3,1)
            ptmm(3, 1, 0)
            ptmm(3, 1, 1)
            ptout(3, 1)
            # tanh(b0) bias derives (=0) from the last cole copy so the
            # scheduler cannot slot it ahead of the pipeline ACT copies.
            biasb0 = big.tile([128, 1], bf16, tag="biasb0")
            nc.vector.tensor_scalar_mul(biasb0, coleTs[1][:, 3, 0:1], 0.0)
            outwrite(0, nc.sync, biasb0)
            # b1 tail: tap3 split across DVE (ahead of stats) + GPSIMD
            tap(1, 3, nc.vector, 0, 32)
            tap(1, 3, nc.gpsimd, 32, C)
            r1 = statsfin(1)
            yfin(1, *r1, nc.vector, 0, 32)
            yfin(1, *r1, nc.gpsimd, 32, C)
            outwrite(1, nc.sync, zerob_sb, split=True)

    nc.compile()
    return nc


def _get_module():
    global _NC
    if _NC is None:
        _NC = _build_module()
    return _NC


def _make_in_maps(A, X, h):
    import ml_dtypes

    bf16 = ml_dtypes.bfloat16
    # AT16[b, c, p, jj, n] = A[b, n, 128*(2c+jj)+p] / 32
    AT = A.transpose(0, 2, 1).reshape(B, 4, 2, 128, N).transpose(0, 1, 3, 2, 4)
    AT16 = (AT / np.float32(32.0)).astype(np.float16)

    Xs = X.astype(np.float32).sum(-1)  # [B, N] (fp32: the fp16 pre-cast cost ~6% of the error budget)
    Xs16 = Xs.astype(np.float16).reshape(B, 8, 128)
    Xsbf = Xs.astype(bf16).view(np.uint16).reshape(B, 8, 128)

    # HBT[p, i, c, t] = h[i, c, 128t+p]
    HBT = np.ascontiguousarray(
        h.reshape(K1, C, 8, 128).transpose(3, 0, 1, 2)
    ).astype(bf16)

    # Host LN covariance moments: HM[n, i] = mean_c h[i,c,n];
    # KV[n, i, j] = mean_c h_i h_j - HM_i HM_j  (so var is a single
    # quadratic form in the taps, no mu^2 step on device)
    hf = h.astype(np.float64)
    HMF64 = hf.mean(axis=1).T  # [N, K1]
    HMF = HMF64.astype(np.float32)
    M2F = (
        np.einsum("icn,jcn->nij", hf, hf) / C
        - HMF64[:, :, None] * HMF64[:, None, :]
    ).astype(np.float32)  # [N, 4, 4]
    M2F = M2F.reshape(N, K1 * K1)
    BF = np.concatenate(
        [
            M2F.reshape(8, 128, 16).transpose(1, 0, 2).reshape(128, 128),
            HMF.reshape(8, 128, K1).transpose(1, 0, 2).reshape(128, 32),
        ],
        axis=1,
    )
    BF = np.ascontiguousarray(BF, dtype=np.float32)

    in_maps = []
    for core in range(NCORES):
        sl = slice(BPC * core, BPC * (core + 1))
        BXBF = np.zeros((128, 354), dtype=np.float16)
        BXBF[:, 0:16] = Xs16[sl].transpose(2, 0, 1).reshape(128, 16)
        BXBF[:, 16:32] = (
            Xsbf[sl].transpose(2, 0, 1).reshape(128, 16).view(np.float16)
        )
        BXBF[0, 32] = 1.0
        BXBF[64, 32] = 1.0
        BXBF = BXBF.view(np.int16).copy()
        BXBF[:, 34:354] = BF.view(np.int16)
        in_maps.append(
            {
                "AT16": np.ascontiguousarray(AT16[sl]),
                "BXBF": np.ascontiguousarray(BXBF),
                "HBT": HBT,
            }
        )
    return in_maps


def _unpermute_out(raw):
    # raw [BPC, 128, C, 8] -> [BPC, N, C] with n = 128t + p
    return np.ascontiguousarray(
        np.asarray(raw).transpose(0, 3, 1, 2).reshape(BPC, N, C)
    )


def _numpy_fallback(A, X, h, ln_gamma, ln_beta):
    Xs = X.sum(-1)
    p = Xs
    powers = [Xs]
    for _ in range(K1 - 1):
        p = np.einsum("bnm,bm->bn", A, p)
        powers.append(p)
    P = np.stack(powers)
    Y = np.einsum("icn,ibn->bnc", h, P)
    mu = Y.mean(axis=-1, keepdims=True)
    var = Y.var(axis=-1, keepdims=True)
    Yn = (Y - mu) / np.sqrt(var + LN_EPS) * ln_gamma + ln_beta
    return np.tanh(Yn).astype(np.float32)


def _run(A, X, h, ln_gamma, ln_beta, trace=False):
    A = np.ascontiguousarray(np.asarray(A, dtype=np.float32))
    X = np.ascontiguousarray(np.asarray(X, dtype=np.float32))
    h = np.ascontiguousarray(np.asarray(h, dtype=np.float32))
    g = np.asarray(ln_gamma, dtype=np.float32)
    be = np.asarray(ln_beta, dtype=np.float32)

    if not (np.all(g == 1.0) and np.all(be == 0.0)):
        # device kernel folds the (identity) affine away; anything else is
        # handled on host
        return _numpy_fallback(A, X, h, g, be), None

    from concourse import bass_utils

    nc = _get_module()
    res = bass_utils.run_bass_kernel_spmd(
        nc, _make_in_maps(A, X, h), core_ids=list(range(NCORES)), trace=trace
    )
    out = np.concatenate(
        [_unpermute_out(r["OUT"]) for r in res.results], axis=0
    )
    return out.astype(np.float32, copy=False), res.exec_time_ns


def kernel(A, X, h, ln_gamma, ln_beta):
    out, _ = _run(A, X, h, ln_gamma, ln_beta, trace=False)
    return out


def kernel_profiled(A, X, h, ln_gamma, ln_beta):
    return _run(A, X, h, ln_gamma, ln_beta, trace=True)
